# revision 25
# baseline (speedup 1.0000x reference)
"""Causal single-head attention (B=8, T=4096, C=1024, H=128) on 8 TRN2
NeuronCores, data-parallel over batch: core b computes batch element b.

Per core: x [T, C] f32 and Wq/Wk/Wv [C, H] f32 (replicated) -> out [T, H] f32.

Hybrid kernel: the baseline's sequential per-chunk emission (load ->
transpose -> project -> attention), which pipelines naturally through the
per-engine in-order queues, plus three targeted fixes:
  * off-diagonal key blocks processed in PAIRS: two score matmuls into a
    2-bank PSUM tile, ONE exp activation for both (halves Act-engine
    instruction overhead: 144 -> 88 activations)
  * PV matmuls lag the exp by one pair, so the PE streams the next pair's
    scores while Act works instead of stalling every key block
  * stores deferred two chunks and issued on the sync queue ahead of the
    transposes (operands long ready -> they never block the queue);
    diagonal mask-multiplies on GpSimd (Pool), off the DVE critical path
Output accumulators: out_psum[q, H+1] += P_j.T @ [v_j | 1] per query block
(ones column yields softmax denominators); two query blocks share one PSUM
bank via the hardware's lazy bank zeroing (only the first group's first
matmul uses start=True; the sibling's first write lands on pending-zero
bytes and overwrites).
"""
import numpy as np

import concourse.bass as bass
import concourse.mybir as mybir
import concourse.tile as tile
from concourse.bass import ts
from contextlib import ExitStack

F16 = mybir.dt.float16
F32 = mybir.dt.float32

B, T, C, H = 8, 4096, 1024, 128

# ---------------------------------------------------------------------------
# Workaround for the walrus build in this container: each TPB instruction may
# carry at most ONE sync-wait ("Too many sync wait commands" otherwise), but
# Tile attaches several. Keep only the last wait per instruction and hoist the
# others onto preceding same-engine NoOps (engines execute their stream in
# order, so the gating semantics are identical). The tail drain gets the same
# treatment.
# ---------------------------------------------------------------------------
_MAX_WAITS = 1
_orig_add_instruction = tile.TileContext._add_instruction


def _split_waits_add_instruction(self, inst):
    si = inst.sync_info
    if (
        si is not None
        and len(si.on_wait) > _MAX_WAITS
        and inst.engine != mybir.EngineType.Unassigned
    ):
        waits = list(si.on_wait)
        extra, keep = waits[:-_MAX_WAITS], waits[-_MAX_WAITS:]
        for w in extra:
            nop = mybir.InstNoOp(
                name=self.nc.get_next_instruction_name(),
                engine=inst.engine,
                ins=[],
                outs=[],
                bass_nofuse=True,
                sync_info=mybir.SyncInfo(on_wait=[w], on_update=[]),
                debug=inst.debug,
            )
            _orig_add_instruction(self, nop)
        inst.sync_info = mybir.SyncInfo(on_wait=keep, on_update=list(si.on_update))
    return _orig_add_instruction(self, inst)


def _split_drain_and_barrier(self, tick_clock, wait_clock):
    nc = self.nc
    probe = nc.sync.nop(nofuse=True, hint="tile_drain_wait_split")
    wait_clock.add_sem_waits(
        probe.ins, tile.ScopedClock({None: tick_clock.global_clock})
    )
    si = probe.ins.sync_info
    waits = list(si.on_wait) if si is not None else []
    if len(waits) > _MAX_WAITS:
        probe.ins.sync_info = mybir.SyncInfo(
            on_wait=waits[:_MAX_WAITS], on_update=list(si.on_update)
        )
        rest = waits[_MAX_WAITS:]
        for i in range(0, len(rest), _MAX_WAITS):
            extra = nc.sync.nop(nofuse=True, hint=f"tile_drain_wait_split_{i}")
            extra.ins.sync_info = mybir.SyncInfo(
                on_wait=rest[i : i + _MAX_WAITS], on_update=[]
            )
    nc.sync.drain()
    nc.all_engine_barrier()
    assert self.sems is not None
    popped = nc._tile_sem_poison_stack.pop()
    assert popped is self._sem_poison
    nc.clear_and_free_semaphores(list(self.sems.allocated().values()))
    nc.all_engine_barrier()


def _apply_tile_patch():
    tile.TileContext._drain_and_barrier = _split_drain_and_barrier
    tile.TileContext._add_instruction = _split_waits_add_instruction


# ---------------------------------------------------------------------------
# Kernel builder
# ---------------------------------------------------------------------------
def build_attention(dtype=F16):
    TB = T // 128   # 32 key blocks
    CB = C // 128   # 8 contraction blocks
    NCH = T // 512  # 8 chunks
    scale = float(H) ** -0.5

    nc = bass.Bass()
    x = nc.dram_tensor("x", [T, C], F32, kind="ExternalInput")
    wq = nc.dram_tensor("wq", [C, H], F32, kind="ExternalInput")
    wk = nc.dram_tensor("wk", [C, H], F32, kind="ExternalInput")
    wv = nc.dram_tensor("wv", [C, H], F32, kind="ExternalInput")
    out = nc.dram_tensor("out", [T, H], F32, kind="ExternalOutput")

    with tile.TileContext(nc) as tc, ExitStack() as ctx:
        const = ctx.enter_context(tc.tile_pool(name="const", bufs=1))
        xsb = ctx.enter_context(tc.tile_pool(name="xsb", bufs=2))
        xtsb = ctx.enter_context(tc.tile_pool(name="xtsb", bufs=2))
        persist = ctx.enter_context(tc.tile_pool(name="persist", bufs=1))
        pP = ctx.enter_context(tc.tile_pool(name="pP", bufs=4))
        pQ = ctx.enter_context(tc.tile_pool(name="pQ", bufs=2))
        osb = ctx.enter_context(tc.tile_pool(name="osb", bufs=12))
        # PSUM: scp pairs 2x2 banks + mm 2 banks + ops 2 banks = 8
        ps_sc = ctx.enter_context(tc.tile_pool(name="ps_sc", bufs=2, space="PSUM"))
        ps_mm = ctx.enter_context(tc.tile_pool(name="ps_mm", bufs=2, space="PSUM"))
        ps_ops = ctx.enter_context(tc.tile_pool(name="ps_ops", bufs=1, space="PSUM"))

        # SWDGE order tuned for startup: x0 tb0, Wq, rest of x0, Wk, Wv
        w16 = {}
        for name in ("q", "k", "v"):
            w16[name] = const.tile(
                [128, CB, H], dtype, tag=f"w{name}", name=f"w16{name}"
            )
        x16_first = xsb.tile([128, 4, C], dtype, tag="x16", name="x16_c0")
        nc.gpsimd.dma_start(x16_first[:, 0, :], x[0:128, :])
        nc.gpsimd.dma_start(
            w16["q"][:], wq[:].rearrange("(cb ci) h -> ci cb h", ci=128)
        )
        for tb in range(1, 4):
            nc.gpsimd.dma_start(
                x16_first[:, tb, :], x[tb * 128 : (tb + 1) * 128, :]
            )
        nc.gpsimd.dma_start(
            w16["k"][:], wk[:].rearrange("(cb ci) h -> ci cb h", ci=128)
        )
        nc.gpsimd.dma_start(
            w16["v"][:], wv[:].rearrange("(cb ci) h -> ci cb h", ci=128)
        )
        # mask16[jl, ql] = 1 if ql >= jl else 0 (transposed-score layout)
        mask16 = const.tile([128, 128], dtype, tag="mask")
        nc.gpsimd.memset(mask16[:], 1.0)
        nc.gpsimd.affine_select(
            out=mask16[:], in_=mask16[:],
            compare_op=mybir.AluOpType.is_ge,
            fill=0.0, base=0, pattern=[[1, 128]], channel_multiplier=-1,
        )

        qT16 = persist.tile([128, T], dtype, tag="qT")
        kT16 = persist.tile([128, T], dtype, tag="kT")
        v16 = persist.tile([128, TB, H + 1], dtype, tag="v")
        nc.vector.memset(v16[:], 1.0)  # ones column survives in col H

        pending_stores = []  # (dram_slice, o32) deferred two chunks

        def flush_stores(n):
            for _ in range(min(n, len(pending_stores))):
                dst, o32 = pending_stores.pop(0)
                nc.sync.dma_start(dst, o32)

        x16_t = {0: x16_first}
        xt16_t = {}

        def emit_load(cc):
            x16_t[cc] = xsb.tile(
                [128, 4, C], dtype, tag="x16", name=f"x16_c{cc}"
            )
            nc.gpsimd.dma_start(
                x16_t[cc][:],
                x[cc * 512 : (cc + 1) * 512, :].rearrange(
                    "(tb ti) c -> ti tb c", ti=128
                ),
            )

        def emit_transposes(cc):
            xt16_t[cc] = xtsb.tile(
                [128, CB, 512], dtype, tag="xt16", name=f"xt16_c{cc}"
            )
            for tb in range(4):
                nc.sync.dma_start(
                    xt16_t[cc][:, :, ts(tb, 128)], x16_t[cc][:, tb, :],
                    transpose=True,
                )

        # prologue: chunk 0+1 data staged ahead of the main loop
        emit_transposes(0)
        emit_load(1)
        emit_transposes(1)

        for c in range(NCH):
            t0 = c * 512
            # stores of chunk c-2: operands ready long ago, so they never
            # stall the sync queue ahead of the transposes
            flush_stores(4)
            # data staging one chunk ahead: load c+2, transpose c+1
            if c + 2 < NCH:
                emit_load(c + 2)
            if c + 1 < NCH:
                emit_transposes(c + 1)
            xt16 = xt16_t[c]
            if c == 0:
                # per-t-block so the first matmuls start after the first
                # transpose instead of all four
                for tb in range(4):
                    for name, dstT in (("q", qT16), ("k", kT16)):
                        pp = ps_mm.tile(
                            [128, 128], F32, tag="mm", name=f"pp0_{name}_{tb}"
                        )
                        for cb in range(CB):
                            nc.tensor.matmul(
                                pp[:], w16[name][:, cb, :],
                                xt16[:, cb, ts(tb, 128)],
                                start=(cb == 0), stop=(cb == CB - 1),
                            )
                        nc.vector.tensor_copy(
                            dstT[:, tb * 128 : (tb + 1) * 128], pp[:]
                        )
            else:
                for name, dstT in (("q", qT16), ("k", kT16)):
                    pp = ps_mm.tile([128, 512], F32, tag="mm")
                    for cb in range(CB):
                        nc.tensor.matmul(
                            pp[:], w16[name][:, cb, :], xt16[:, cb, :],
                            start=(cb == 0), stop=(cb == CB - 1),
                        )
                    nc.vector.tensor_copy(dstT[:, t0 : t0 + 512], pp[:])
            for tb in range(4):
                pv = ps_mm.tile([128, 128], F32, tag="mm")
                for cb in range(CB):
                    nc.tensor.matmul(
                        pv[:], xt16[:, cb, ts(tb, 128)], w16["v"][:, cb, :],
                        start=(cb == 0), stop=(cb == CB - 1),
                    )
                nc.vector.tensor_copy(v16[:, c * 4 + tb, 0:H], pv[:])

            # phase B: attention for queries [t0, t0+512)
            ops01 = ps_ops.tile([128, 2, H + 1], F32, tag="o01", name=f"o01_{c}")
            ops23 = ps_ops.tile([128, 2, H + 1], F32, tag="o23", name=f"o23_{c}")
            opsr = [
                ops01[:, 0, :], ops01[:, 1, :], ops23[:, 0, :], ops23[:, 1, :]
            ]

            def emit_pv(p16pair, m):
                """PV matmuls for off-diagonal pair m (j = 2m, 2m+1).

                Each ops bank hosts TWO accumulation regions (qb even/odd).
                Only the even region's first matmul starts the bank (its
                lazy-zero marks both regions); the odd region's first write
                lands on pending-zero bytes and overwrites."""
                for qb in range(4):
                    for jt in range(2):
                        j = 2 * m + jt
                        nc.tensor.matmul(
                            opsr[qb], p16pair[:, jt, ts(qb, 128)],
                            v16[:, j, :],
                            start=(j == 0 and qb % 2 == 0), stop=False,
                        )

            # ---- off-diagonal pairs, PV lagging by one pair ----
            npairs = 2 * c
            prev = None
            for m in range(npairs):
                sc = ps_sc.tile([128, 2, 512], F32, tag="sc", name=f"sc_{c}_{m}")
                for jt in range(2):
                    nc.tensor.matmul(
                        sc[:, jt, :], kT16[:, ts(2 * m + jt, 128)],
                        qT16[:, t0 : t0 + 512],
                        start=True, stop=True,
                    )
                p16 = pP.tile([128, 2, 512], dtype, tag="p", name=f"p_{c}_{m}")
                nc.scalar.activation(
                    p16[:], sc[:],
                    mybir.ActivationFunctionType.Exp, scale=scale,
                )
                if prev is not None:
                    emit_pv(*prev)
                prev = (p16, m)

            # ---- diagonal blocks j = 4c+d ----
            pq = pQ.tile([128, 4, 512], dtype, tag="pq", name=f"pq_{c}")
            scA = ps_sc.tile([128, 2, 512], F32, tag="sc", name=f"scA_{c}")
            nc.tensor.matmul(
                scA[:, 0, :], kT16[:, ts(4 * c, 128)], qT16[:, t0 : t0 + 512],
                start=True, stop=True,
            )
            nc.tensor.matmul(
                scA[:, 1, 128:512], kT16[:, ts(4 * c + 1, 128)],
                qT16[:, t0 + 128 : t0 + 512],
                start=True, stop=True,
            )
            if prev is not None:
                emit_pv(*prev)
                prev = None
            for d in range(2):
                q_lo = d * 128
                nc.scalar.activation(
                    pq[:, d, q_lo:512], scA[:, d, q_lo:512],
                    mybir.ActivationFunctionType.Exp, scale=scale,
                )
                nc.gpsimd.tensor_mul(
                    pq[:, d, ts(d, 128)], pq[:, d, ts(d, 128)], mask16[:]
                )
            scB = ps_sc.tile([128, 2, 512], F32, tag="sc", name=f"scB_{c}")
            for i, d in enumerate((2, 3)):
                q_lo = d * 128
                nc.tensor.matmul(
                    scB[:, i, q_lo:512], kT16[:, ts(4 * c + d, 128)],
                    qT16[:, t0 + q_lo : t0 + 512],
                    start=True, stop=True,
                )
            for i, d in enumerate((2, 3)):
                q_lo = d * 128
                nc.scalar.activation(
                    pq[:, d, q_lo:512], scB[:, i, q_lo:512],
                    mybir.ActivationFunctionType.Exp, scale=scale,
                )
                nc.gpsimd.tensor_mul(
                    pq[:, d, ts(d, 128)], pq[:, d, ts(d, 128)], mask16[:]
                )
            # diagonal PV (banks close at the odd region's final matmul),
            # then normalize; stores are deferred two chunks
            for qb in range(4):
                for d in range(qb + 1):
                    nc.tensor.matmul(
                        opsr[qb], pq[:, d, ts(qb, 128)], v16[:, 4 * c + d, :],
                        start=(c == 0 and d == 0 and qb % 2 == 0),
                        stop=(d == qb and qb % 2 == 1),
                    )
            for qb in range(4):
                rec = osb.tile([128, 1], F32, tag="rec", bufs=4)
                nc.vector.reciprocal(rec[:], opsr[qb][:, H : H + 1])
                o32 = osb.tile([128, H], F32, tag="o32")
                nc.vector.tensor_scalar_mul(o32[:], opsr[qb][:, 0:H], rec[:])
                pending_stores.append(
                    (out[t0 + qb * 128 : t0 + (qb + 1) * 128, :], o32[:])
                )

        flush_stores(len(pending_stores))

    return nc


_NC_CACHE = None


def _get_nc():
    global _NC_CACHE
    if _NC_CACHE is None:
        _apply_tile_patch()
        _NC_CACHE = build_attention()
    return _NC_CACHE


def _install_ntff_hook_shim():
    """antenv.axon_hooks is absent on this image, which makes
    run_bass_kernel_spmd(trace=True) crash instead of degrading. Provide the
    module and register the ctypes NTFF hook the boot script would have."""
    import sys, types
    try:
        import antenv.axon_hooks  # noqa: F401
        return
    except ImportError:
        pass
    try:
        import antenv
    except ImportError:
        return
    mod = types.ModuleType("antenv.axon_hooks")
    _hook = [None]
    mod.set_axon_ntff_profile_hook = lambda h: _hook.__setitem__(0, h)
    mod.get_axon_ntff_profile_hook = lambda: _hook[0]
    sys.modules["antenv.axon_hooks"] = mod
    antenv.axon_hooks = mod
    try:
        from trn_agent_boot.trn_boot import _ntff_profile_via_ctypes
        mod.set_axon_ntff_profile_hook(
            _ntff_profile_via_ctypes("/opt/axon/libaxon_pjrt.so")
        )
    except Exception:
        pass


def kernel(x, Wk, Wq, Wv, trace=False):
    """Full inputs in, full output out. Shards batch across the 8 cores."""
    from concourse.bass_utils import run_bass_kernel_spmd

    if trace:
        _install_ntff_hook_shim()

    x = np.ascontiguousarray(np.asarray(x, dtype=np.float32))
    Wk = np.ascontiguousarray(np.asarray(Wk, dtype=np.float32))
    Wq = np.ascontiguousarray(np.asarray(Wq, dtype=np.float32))
    Wv = np.ascontiguousarray(np.asarray(Wv, dtype=np.float32))
    assert x.shape == (B, T, C), x.shape

    nc = _get_nc()
    in_maps = [
        {"x": x[b], "wq": Wq, "wk": Wk, "wv": Wv} for b in range(B)
    ]
    res = run_bass_kernel_spmd(nc, in_maps, core_ids=list(range(B)), trace=trace)
    outp = np.stack([res.results[b]["out"] for b in range(B)], axis=0)
    if trace:
        return outp, res.exec_time_ns
    return outp


# revision 27
# speedup vs baseline: 1.1584x; 1.1584x over previous
"""Causal single-head attention (B=8, T=4096, C=1024, H=128) on 8 TRN2
NeuronCores, data-parallel over batch: core b computes batch element b.

Per core: x [T, C] f32 and Wq/Wk/Wv [C, H] f32 (replicated) -> out [T, H] f32.

Hybrid kernel: the baseline's sequential per-chunk emission (load ->
transpose -> project -> attention), which pipelines naturally through the
per-engine in-order queues, plus three targeted fixes:
  * off-diagonal key blocks processed in PAIRS: two score matmuls into a
    2-bank PSUM tile, ONE exp activation for both (halves Act-engine
    instruction overhead: 144 -> 88 activations)
  * PV matmuls lag the exp by one pair, so the PE streams the next pair's
    scores while Act works instead of stalling every key block
  * stores deferred two chunks and issued on the sync queue ahead of the
    transposes (operands long ready -> they never block the queue);
    diagonal mask-multiplies on GpSimd (Pool), off the DVE critical path
Output accumulators: out_psum[q, H+1] += P_j.T @ [v_j | 1] per query block
(ones column yields softmax denominators); two query blocks share one PSUM
bank via the hardware's lazy bank zeroing (only the first group's first
matmul uses start=True; the sibling's first write lands on pending-zero
bytes and overwrites).
"""
import numpy as np

import concourse.bass as bass
import concourse.mybir as mybir
import concourse.tile as tile
from concourse.bass import ts
from contextlib import ExitStack

F16 = mybir.dt.float16
F32 = mybir.dt.float32

B, T, C, H = 8, 4096, 1024, 128

# ---------------------------------------------------------------------------
# Workaround for the walrus build in this container: each TPB instruction may
# carry at most ONE sync-wait ("Too many sync wait commands" otherwise), but
# Tile attaches several. Keep only the last wait per instruction and hoist the
# others onto preceding same-engine NoOps (engines execute their stream in
# order, so the gating semantics are identical). The tail drain gets the same
# treatment.
# ---------------------------------------------------------------------------
_MAX_WAITS = 1
_orig_add_instruction = tile.TileContext._add_instruction


def _split_waits_add_instruction(self, inst):
    si = inst.sync_info
    if (
        si is not None
        and len(si.on_wait) > _MAX_WAITS
        and inst.engine != mybir.EngineType.Unassigned
    ):
        waits = list(si.on_wait)
        extra, keep = waits[:-_MAX_WAITS], waits[-_MAX_WAITS:]
        for w in extra:
            nop = mybir.InstNoOp(
                name=self.nc.get_next_instruction_name(),
                engine=inst.engine,
                ins=[],
                outs=[],
                bass_nofuse=True,
                sync_info=mybir.SyncInfo(on_wait=[w], on_update=[]),
                debug=inst.debug,
            )
            _orig_add_instruction(self, nop)
        inst.sync_info = mybir.SyncInfo(on_wait=keep, on_update=list(si.on_update))
    return _orig_add_instruction(self, inst)


def _split_drain_and_barrier(self, tick_clock, wait_clock):
    nc = self.nc
    probe = nc.sync.nop(nofuse=True, hint="tile_drain_wait_split")
    wait_clock.add_sem_waits(
        probe.ins, tile.ScopedClock({None: tick_clock.global_clock})
    )
    si = probe.ins.sync_info
    waits = list(si.on_wait) if si is not None else []
    if len(waits) > _MAX_WAITS:
        probe.ins.sync_info = mybir.SyncInfo(
            on_wait=waits[:_MAX_WAITS], on_update=list(si.on_update)
        )
        rest = waits[_MAX_WAITS:]
        for i in range(0, len(rest), _MAX_WAITS):
            extra = nc.sync.nop(nofuse=True, hint=f"tile_drain_wait_split_{i}")
            extra.ins.sync_info = mybir.SyncInfo(
                on_wait=rest[i : i + _MAX_WAITS], on_update=[]
            )
    nc.sync.drain()
    nc.all_engine_barrier()
    assert self.sems is not None
    popped = nc._tile_sem_poison_stack.pop()
    assert popped is self._sem_poison
    nc.clear_and_free_semaphores(list(self.sems.allocated().values()))
    nc.all_engine_barrier()


def _apply_tile_patch():
    tile.TileContext._drain_and_barrier = _split_drain_and_barrier
    tile.TileContext._add_instruction = _split_waits_add_instruction


# ---------------------------------------------------------------------------
# Kernel builder
# ---------------------------------------------------------------------------
def build_attention(dtype=F16):
    TB = T // 128   # 32 key blocks
    CB = C // 128   # 8 contraction blocks
    NCH = T // 512  # 8 chunks
    scale = float(H) ** -0.5

    nc = bass.Bass()
    x = nc.dram_tensor("x", [T, C], F32, kind="ExternalInput")
    wq = nc.dram_tensor("wq", [C, H], F32, kind="ExternalInput")
    wk = nc.dram_tensor("wk", [C, H], F32, kind="ExternalInput")
    wv = nc.dram_tensor("wv", [C, H], F32, kind="ExternalInput")
    out = nc.dram_tensor("out", [T, H], F32, kind="ExternalOutput")

    with tile.TileContext(nc) as tc, ExitStack() as ctx:
        const = ctx.enter_context(tc.tile_pool(name="const", bufs=1))
        xsb = ctx.enter_context(tc.tile_pool(name="xsb", bufs=2))
        xtsb = ctx.enter_context(tc.tile_pool(name="xtsb", bufs=2))
        persist = ctx.enter_context(tc.tile_pool(name="persist", bufs=1))
        pP = ctx.enter_context(tc.tile_pool(name="pP", bufs=4))
        pQ = ctx.enter_context(tc.tile_pool(name="pQ", bufs=2))
        osb = ctx.enter_context(tc.tile_pool(name="osb", bufs=12))
        # PSUM: scp pairs 2x2 banks + mm 2 banks + ops 2 banks = 8
        ps_sc = ctx.enter_context(tc.tile_pool(name="ps_sc", bufs=2, space="PSUM"))
        ps_mm = ctx.enter_context(tc.tile_pool(name="ps_mm", bufs=2, space="PSUM"))
        ps_ops = ctx.enter_context(tc.tile_pool(name="ps_ops", bufs=1, space="PSUM"))

        # SWDGE order tuned for startup: x0 tb0, Wq, rest of x0, Wk, Wv
        w16 = {}
        for name in ("q", "k", "v"):
            w16[name] = const.tile(
                [128, CB, H], dtype, tag=f"w{name}", name=f"w16{name}"
            )
        x16_first = xsb.tile([128, 4, C], dtype, tag="x16", name="x16_c0")
        nc.gpsimd.dma_start(x16_first[:, 0, :], x[0:128, :])
        nc.gpsimd.dma_start(
            w16["q"][:], wq[:].rearrange("(cb ci) h -> ci cb h", ci=128)
        )
        for tb in range(1, 4):
            nc.gpsimd.dma_start(
                x16_first[:, tb, :], x[tb * 128 : (tb + 1) * 128, :]
            )
        nc.gpsimd.dma_start(
            w16["k"][:], wk[:].rearrange("(cb ci) h -> ci cb h", ci=128)
        )
        nc.gpsimd.dma_start(
            w16["v"][:], wv[:].rearrange("(cb ci) h -> ci cb h", ci=128)
        )
        # mask16[jl, ql] = 1 if ql >= jl else 0 (transposed-score layout)
        mask16 = const.tile([128, 128], dtype, tag="mask")
        nc.gpsimd.memset(mask16[:], 1.0)
        nc.gpsimd.affine_select(
            out=mask16[:], in_=mask16[:],
            compare_op=mybir.AluOpType.is_ge,
            fill=0.0, base=0, pattern=[[1, 128]], channel_multiplier=-1,
        )

        qT16 = persist.tile([128, T], dtype, tag="qT")
        kT16 = persist.tile([128, T], dtype, tag="kT")
        v16 = persist.tile([128, TB, H + 1], dtype, tag="v")
        nc.vector.memset(v16[:], 1.0)  # ones column survives in col H

        pending_stores = []  # (dram_slice, o32) deferred two chunks

        def flush_stores(n):
            for _ in range(min(n, len(pending_stores))):
                dst, o32 = pending_stores.pop(0)
                nc.sync.dma_start(dst, o32)

        for c in range(NCH):
            t0 = c * 512
            # stores of chunk c-2: operands ready long ago, so they never
            # stall the sync queue ahead of the transposes
            flush_stores(4)
            # phase A: load, transpose, project. The Pool queue carries ONLY
            # loads, so each chunk's load dispatches as soon as the previous
            # one's descriptors are generated — a free ~1-chunk DMA lead.
            if c == 0:
                x16 = x16_first
            else:
                x16 = xsb.tile([128, 4, C], dtype, tag="x16", name=f"x16_c{c}")
                nc.gpsimd.dma_start(
                    x16[:],
                    x[t0 : t0 + 512, :].rearrange("(tb ti) c -> ti tb c", ti=128),
                )
            xt16 = xtsb.tile([128, CB, 512], dtype, tag="xt16")
            for tb in range(4):
                nc.sync.dma_start(
                    xt16[:, :, ts(tb, 128)], x16[:, tb, :], transpose=True
                )
            if c == 0:
                # per-t-block so the first matmuls start after the first
                # transpose instead of all four
                for tb in range(4):
                    for name, dstT in (("q", qT16), ("k", kT16)):
                        pp = ps_mm.tile(
                            [128, 128], F32, tag="mm", name=f"pp0_{name}_{tb}"
                        )
                        for cb in range(CB):
                            nc.tensor.matmul(
                                pp[:], w16[name][:, cb, :],
                                xt16[:, cb, ts(tb, 128)],
                                start=(cb == 0), stop=(cb == CB - 1),
                            )
                        nc.vector.tensor_copy(
                            dstT[:, tb * 128 : (tb + 1) * 128], pp[:]
                        )
            else:
                for name, dstT in (("q", qT16), ("k", kT16)):
                    pp = ps_mm.tile([128, 512], F32, tag="mm")
                    for cb in range(CB):
                        nc.tensor.matmul(
                            pp[:], w16[name][:, cb, :], xt16[:, cb, :],
                            start=(cb == 0), stop=(cb == CB - 1),
                        )
                    nc.vector.tensor_copy(dstT[:, t0 : t0 + 512], pp[:])
            for tb in range(4):
                pv = ps_mm.tile([128, 128], F32, tag="mm")
                for cb in range(CB):
                    nc.tensor.matmul(
                        pv[:], xt16[:, cb, ts(tb, 128)], w16["v"][:, cb, :],
                        start=(cb == 0), stop=(cb == CB - 1),
                    )
                nc.vector.tensor_copy(v16[:, c * 4 + tb, 0:H], pv[:])

            # phase B: attention for queries [t0, t0+512)
            ops01 = ps_ops.tile([128, 2, H + 1], F32, tag="o01", name=f"o01_{c}")
            ops23 = ps_ops.tile([128, 2, H + 1], F32, tag="o23", name=f"o23_{c}")
            opsr = [
                ops01[:, 0, :], ops01[:, 1, :], ops23[:, 0, :], ops23[:, 1, :]
            ]

            def emit_pv(p16pair, m):
                """PV matmuls for off-diagonal pair m (j = 2m, 2m+1).

                Each ops bank hosts TWO accumulation regions (qb even/odd).
                Only the even region's first matmul starts the bank (its
                lazy-zero marks both regions); the odd region's first write
                lands on pending-zero bytes and overwrites."""
                for qb in range(4):
                    for jt in range(2):
                        j = 2 * m + jt
                        nc.tensor.matmul(
                            opsr[qb], p16pair[:, jt, ts(qb, 128)],
                            v16[:, j, :],
                            start=(j == 0 and qb % 2 == 0), stop=False,
                        )

            # ---- off-diagonal pairs, PV lagging by one pair ----
            npairs = 2 * c
            prev = None
            for m in range(npairs):
                sc = ps_sc.tile([128, 2, 512], F32, tag="sc", name=f"sc_{c}_{m}")
                for jt in range(2):
                    nc.tensor.matmul(
                        sc[:, jt, :], kT16[:, ts(2 * m + jt, 128)],
                        qT16[:, t0 : t0 + 512],
                        start=True, stop=True,
                    )
                p16 = pP.tile([128, 2, 512], dtype, tag="p", name=f"p_{c}_{m}")
                nc.scalar.activation(
                    p16[:], sc[:],
                    mybir.ActivationFunctionType.Exp, scale=scale,
                )
                if prev is not None:
                    emit_pv(*prev)
                prev = (p16, m)

            # ---- diagonal blocks j = 4c+d ----
            pq = pQ.tile([128, 4, 512], dtype, tag="pq", name=f"pq_{c}")
            scA = ps_sc.tile([128, 2, 512], F32, tag="sc", name=f"scA_{c}")
            nc.tensor.matmul(
                scA[:, 0, :], kT16[:, ts(4 * c, 128)], qT16[:, t0 : t0 + 512],
                start=True, stop=True,
            )
            nc.tensor.matmul(
                scA[:, 1, 128:512], kT16[:, ts(4 * c + 1, 128)],
                qT16[:, t0 + 128 : t0 + 512],
                start=True, stop=True,
            )
            if prev is not None:
                emit_pv(*prev)
                prev = None
            for d in range(2):
                q_lo = d * 128
                nc.scalar.activation(
                    pq[:, d, q_lo:512], scA[:, d, q_lo:512],
                    mybir.ActivationFunctionType.Exp, scale=scale,
                )
                nc.vector.tensor_mul(
                    pq[:, d, ts(d, 128)], pq[:, d, ts(d, 128)], mask16[:]
                )
            scB = ps_sc.tile([128, 2, 512], F32, tag="sc", name=f"scB_{c}")
            for i, d in enumerate((2, 3)):
                q_lo = d * 128
                nc.tensor.matmul(
                    scB[:, i, q_lo:512], kT16[:, ts(4 * c + d, 128)],
                    qT16[:, t0 + q_lo : t0 + 512],
                    start=True, stop=True,
                )
            for i, d in enumerate((2, 3)):
                q_lo = d * 128
                nc.scalar.activation(
                    pq[:, d, q_lo:512], scB[:, i, q_lo:512],
                    mybir.ActivationFunctionType.Exp, scale=scale,
                )
                nc.vector.tensor_mul(
                    pq[:, d, ts(d, 128)], pq[:, d, ts(d, 128)], mask16[:]
                )
            # diagonal PV (banks close at the odd region's final matmul),
            # then normalize; stores are deferred two chunks
            for qb in range(4):
                for d in range(qb + 1):
                    nc.tensor.matmul(
                        opsr[qb], pq[:, d, ts(qb, 128)], v16[:, 4 * c + d, :],
                        start=(c == 0 and d == 0 and qb % 2 == 0),
                        stop=(d == qb and qb % 2 == 1),
                    )
            for qb in range(4):
                rec = osb.tile([128, 1], F32, tag="rec", bufs=4)
                nc.vector.reciprocal(rec[:], opsr[qb][:, H : H + 1])
                o32 = osb.tile([128, H], F32, tag="o32")
                nc.vector.tensor_scalar_mul(o32[:], opsr[qb][:, 0:H], rec[:])
                pending_stores.append(
                    (out[t0 + qb * 128 : t0 + (qb + 1) * 128, :], o32[:])
                )

        flush_stores(len(pending_stores))

    return nc


_NC_CACHE = None


def _get_nc():
    global _NC_CACHE
    if _NC_CACHE is None:
        _apply_tile_patch()
        _NC_CACHE = build_attention()
    return _NC_CACHE


def _install_ntff_hook_shim():
    """antenv.axon_hooks is absent on this image, which makes
    run_bass_kernel_spmd(trace=True) crash instead of degrading. Provide the
    module and register the ctypes NTFF hook the boot script would have."""
    import sys, types
    try:
        import antenv.axon_hooks  # noqa: F401
        return
    except ImportError:
        pass
    try:
        import antenv
    except ImportError:
        return
    mod = types.ModuleType("antenv.axon_hooks")
    _hook = [None]
    mod.set_axon_ntff_profile_hook = lambda h: _hook.__setitem__(0, h)
    mod.get_axon_ntff_profile_hook = lambda: _hook[0]
    sys.modules["antenv.axon_hooks"] = mod
    antenv.axon_hooks = mod
    try:
        from trn_agent_boot.trn_boot import _ntff_profile_via_ctypes
        mod.set_axon_ntff_profile_hook(
            _ntff_profile_via_ctypes("/opt/axon/libaxon_pjrt.so")
        )
    except Exception:
        pass


def kernel(x, Wk, Wq, Wv, trace=False):
    """Full inputs in, full output out. Shards batch across the 8 cores."""
    from concourse.bass_utils import run_bass_kernel_spmd

    if trace:
        _install_ntff_hook_shim()

    x = np.ascontiguousarray(np.asarray(x, dtype=np.float32))
    Wk = np.ascontiguousarray(np.asarray(Wk, dtype=np.float32))
    Wq = np.ascontiguousarray(np.asarray(Wq, dtype=np.float32))
    Wv = np.ascontiguousarray(np.asarray(Wv, dtype=np.float32))
    assert x.shape == (B, T, C), x.shape

    nc = _get_nc()
    in_maps = [
        {"x": x[b], "wq": Wq, "wk": Wk, "wv": Wv} for b in range(B)
    ]
    res = run_bass_kernel_spmd(nc, in_maps, core_ids=list(range(B)), trace=trace)
    outp = np.stack([res.results[b]["out"] for b in range(B)], axis=0)
    if trace:
        return outp, res.exec_time_ns
    return outp
